# revision 8
# baseline (speedup 1.0000x reference)
"""Trainium2 Bass kernel for CustomISTFT (N_FFT=4096, HOP=1024, T=4096 frames).

Per core (frames sharded 512/core across 8 cores):
  Cooley-Tukey split of the 4096-point inverse DFT: j = 64*j1 + j2,
  n = m1 + 64*m2.  Stage 1 contracts j1 per j2-column (Hermitian
  extension + twiddle folded into host-built weights); the (m1<->j2)
  corner turn bounces through DRAM with 2KB-contiguous descriptors.
  Stage 2 contracts (c,j2) and performs the windowed overlap-add
  DIRECTLY IN PSUM: for each m1, four accumulating matmuls write
  r-shifted output ranges P[:, r:r+T] of one 2-bank PSUM tile, so the
  OLA needs no vector-engine pass and no second corner turn; the
  (par,ih)-partition result is written out as-is and transposed on the
  host.  z is pre-gathered and cast to bf16 on the host (one contiguous
  load); the imaginary channel is the cheap rank-2 b0/b2048 path.
  DMA issue is spread across the SP/Activation HWDGE queues and the
  gpsimd SWDGE queue.
"""

import numpy as np
import ml_dtypes

N_FFT = 4096
HOP = 1024
FREQ = 2049
T_FRAMES = 4096
N_CORES = 8
T_CORE = T_FRAMES // N_CORES  # 512
L_FULL = (T_FRAMES - 1) * HOP + N_FFT
OUT_LEN = L_FULL - N_FFT

NG = 33  # stage-1 row groups
NU = 32  # stage-1 units (j2 pairs)
SB = T_CORE + 3  # 515
SPAD = 520
SC = (SB + 127) // 128  # 5

_bf16 = ml_dtypes.bfloat16


# ---------------------------------------------------------------- weights
def canonical_rows(j2):
    """(c, k) input rows consumed by the stage-1 call group of column j2.
    None entries are unused (zero-weighted) padding rows."""
    if j2 == 0:
        return [(0, 64 * j1) for j1 in range(33)] + [(1, 64 * j1) for j1 in range(33)]
    if j2 == 32:
        return [(0, 32 + 64 * j1) for j1 in range(32)] + [
            (1, 32 + 64 * j1) for j1 in range(32)
        ]
    if j2 > 32:
        return canonical_rows(64 - j2)
    return (
        [(0, j2 + 64 * j1) for j1 in range(32)]
        + [(1, j2 + 64 * j1) for j1 in range(32)]
        + [(0, (64 - j2) + 64 * j1) for j1 in range(32)]
        + [(1, (64 - j2) + 64 * j1) for j1 in range(32)]
    )


def sigma_of_j2(j2):
    if j2 == 0:
        return 0
    if j2 == 32:
        return 1
    if j2 < 32:
        return 2 * j2
    return 2 * (64 - j2) + 1


PORDER = [0, 32] + list(range(1, 32))  # zin group -> p


def build_weights(window):
    """w1T [128,64,128] bf16 (k-row, j2, out-col), w2rT [128,64,4,16] bf16
    ((c,sigma)-row, m1, r, par*8+ih), wim [8,1024] bf16."""
    win = window.astype(np.float64)
    mu = np.exp(2j * np.pi / 4096)
    w64c = np.exp(2j * np.pi / 64)
    m1v = np.arange(64)

    w1 = np.zeros((64, 128, 128), dtype=np.float32)
    for j2 in range(64):
        coef = {}
        for j1 in range(64):
            k = 64 * j1 + j2
            e = w64c ** (m1v * j1)
            if k <= 2048:
                coef[(0, k)] = coef.get((0, k), 0) + e
                coef[(1, k)] = coef.get((1, k), 0) + 1j * e
            else:
                kr = 4096 - k
                coef[(0, kr)] = coef.get((0, kr), 0) + e
                coef[(1, kr)] = coef.get((1, kr), 0) - 1j * e
        tw = mu ** (m1v * j2)
        rows = canonical_rows(j2)
        assert set(rows) == set(coef.keys())
        for i, key in enumerate(rows):
            v = coef[key] * tw
            w1[j2, i, :64] = v.real.astype(np.float32)
            w1[j2, i, 64:] = v.imag.astype(np.float32)
    w1T = np.ascontiguousarray(w1.transpose(1, 0, 2))

    # stage-2 base weights: rows (c, j2-literal), cols m2; win folded
    m2v = np.arange(64)
    j2v = np.arange(64)
    ang = 2 * np.pi * np.outer(j2v, m2v) / 64
    c = np.cos(ang) / 4096
    s = np.sin(ang) / 4096
    w2 = np.zeros((64, 128, 64), dtype=np.float64)  # [m1, (c,j2), m2]
    for m1 in range(64):
        n = m1 + 64 * m2v
        wn = win[n] * (4096.0 / 3.0)
        w2[m1, :64, :] = c * wn[None, :]
        w2[m1, 64:, :] = -s * wn[None, :]

    # repack to (c,sigma) rows x (m1, r, q=par*8+ih) with m2 = 2*(ih+8r)+par
    qv = np.arange(16)
    rv = np.arange(4)
    m2_rq = 2 * ((qv[None, :] % 8) + 8 * rv[:, None]) + qv[None, :] // 8  # [4,16]
    w2rT = np.zeros((128, 64, 4, 16), dtype=np.float64)
    for j2 in range(64):
        sg = sigma_of_j2(j2)
        for cc in range(2):
            # w2[:, cc*64+j2, m2_rq] -> [64 m1, 4, 16]
            w2rT[cc * 64 + sg] = w2[:, cc * 64 + j2, :][:, m2_rq]

    # wim[(2r+par), i] = win[i + 1024 r]/3 * (par == i%2)
    wim = np.zeros((8, 1024), dtype=np.float64)
    iv = np.arange(1024)
    for r in range(4):
        for par in range(2):
            wim[2 * r + par] = (win[iv + 1024 * r] / 3.0) * (iv % 2 == par)
    return w1T.astype(_bf16), w2rT.astype(_bf16), wim.astype(_bf16)


_ZIDX = None


def _z_index_arrays():
    """cidx/kidx [33,128] gather indices into z[c,k,:], pad mask."""
    global _ZIDX
    if _ZIDX is None:
        cidx = np.zeros((NG, 128), dtype=np.int64)
        kidx = np.zeros((NG, 128), dtype=np.int64)
        pad = np.zeros((NG, 128), dtype=bool)
        for g, p in enumerate(PORDER):
            rows = canonical_rows(p)
            for i in range(128):
                if i < len(rows):
                    cidx[g, i], kidx[g, i] = rows[i]
                else:
                    pad[g, i] = True
        _ZIDX = (cidx, kidx, pad)
    return _ZIDX


def prep_z_core(z_core):
    """z_core [2, 2049, T] f32 -> zin [33, 128, T] bf16 (canonical rows)."""
    cidx, kidx, pad = _z_index_arrays()
    zin = z_core[cidx, kidx]  # [33, 128, T]
    zin[pad] = 0.0
    return zin.astype(_bf16)


# ---------------------------------------------------------------- device program
def emit_kernel(tc, outre_ap, outim_ap, zin_ap, w1_ap, w2_ap, wim_ap, T):
    import concourse.mybir as mybir
    from contextlib import ExitStack

    nc = tc.nc
    dt = mybir.dt
    f32, bf16 = dt.float32, dt.bfloat16

    with ExitStack() as ctx:
        const = ctx.enter_context(tc.tile_pool(name="const", bufs=1))
        dram = ctx.enter_context(tc.tile_pool(name="dram", bufs=1, space="DRAM"))
        s1ps = ctx.enter_context(tc.tile_pool(name="s1ps", bufs=2, space="PSUM"))
        s2ps = ctx.enter_context(tc.tile_pool(name="s2ps", bufs=2, space="PSUM"))
        apool = ctx.enter_context(tc.tile_pool(name="aslot", bufs=3))
        xpool = ctx.enter_context(tc.tile_pool(name="xout", bufs=2))
        impool = ctx.enter_context(tc.tile_pool(name="imsb", bufs=2))

        # ---- weights to SBUF (contiguous, host-prelayouted)
        w1_sb = const.tile([128, 64, 128], bf16)
        nc.scalar.dma_start(w1_sb[:], w1_ap[:])
        w2_sb = const.tile([128, 64, 4, 16], bf16)
        nc.scalar.dma_start(w2_sb[:], w2_ap[:])
        wim_sb = const.tile([8, 1024], bf16)
        nc.scalar.dma_start(wim_sb[:], wim_ap[:])
        wz = const.tile([128, 16], bf16)
        nc.vector.memset(wz[:], 0.0)

        # ---- z chunks (5 tiles of <=7 groups)
        zchunks = []
        csizes = [7, 7, 7, 7, 5]
        g0 = 0
        for ci, cs_ in enumerate(csizes):
            zt = const.tile([128, cs_, T], bf16)
            nc.sync.dma_start(
                zt[:], zin_ap[g0 : g0 + cs_].rearrange("g p t -> p g t")
            )
            zchunks.append((g0, cs_, zt))
            g0 += cs_

        def zslice(g):
            for base, n, zt in zchunks:
                if base <= g < base + n:
                    return zt[:, g - base, :]
            raise AssertionError

        adram = dram.tile([2, 64, 64, T], bf16)  # (c, m1, sigma, t)

        # ---- imag channel consts (needs zin group 0 rows 33/65)
        cve = const.tile([1, T], bf16)
        cvo = const.tile([1, T], bf16)
        b0t = const.tile([1, T], bf16)
        b2t = const.tile([1, T], bf16)
        z0 = zchunks[0][2]
        nc.scalar.dma_start(b0t[:], z0[33:34, 0, :])
        nc.scalar.dma_start(b2t[:], z0[65:66, 0, :])
        nc.vector.tensor_add(cve[:], b0t[:], b2t[:])
        nc.vector.tensor_sub(cvo[:], b0t[:], b2t[:])
        cs = const.tile([8, SC * 128], bf16)
        nc.vector.memset(cs[:], 0.0)
        for r in range(4):
            nc.sync.dma_start(cs[2 * r : 2 * r + 1, r : r + T], cve[:])
            nc.sync.dma_start(cs[2 * r + 1 : 2 * r + 2, r : r + T], cvo[:])

        # ---- stage 1: 32 units, each 2 matmuls -> psum [128,1024] -> bf16 ->
        #      A-write (c, m1, sigma-pair contiguous, t)
        def stage1_unit(u):
            if u == 0:
                pairs = [(0, 0), (32, 1)]  # (j2, zin group)
            else:
                pairs = [(u, u + 1), (64 - u, u + 1)]
            ps = s1ps.tile([128, 1024], f32, tag="s1ps")
            for h, (j2, g) in enumerate(pairs):
                nc.tensor.matmul(
                    ps[:, h * T : (h + 1) * T],
                    w1_sb[:, j2, :],
                    zslice(g),
                    start=True,
                    stop=True,
                )
            aslot = apool.tile([128, 1024], bf16, tag="aslot")
            if u % 2 == 0:
                nc.scalar.copy(aslot[:], ps[:])
            else:
                nc.vector.tensor_copy(aslot[:], ps[:])
            dst = adram[:, :, 2 * u : 2 * u + 2, :]
            eng = nc.sync if u % 2 == 0 else nc.scalar
            eng.dma_start(dst, aslot[:].rearrange("p (h t) -> p h t", h=2))

        # ---- imag channel matmuls (interleave early)
        def imag_sc(sc_):
            ips = s1ps.tile([128, 1024], f32, tag="s1ps")
            for half in range(2):
                nc.tensor.matmul(
                    ips[:, half * 512 : (half + 1) * 512],
                    cs[:, sc_ * 128 : (sc_ + 1) * 128],
                    wim_sb[:, half * 512 : (half + 1) * 512],
                    start=True,
                    stop=True,
                )
            it = impool.tile([128, 1024], bf16, tag="imsb")
            nc.vector.tensor_copy(it[:], ips[:])
            nc.sync.dma_start(outim_ap[sc_], it[:])

        for u in range(NU):
            stage1_unit(u)
            if u in (1, 3, 5, 7, 9):
                imag_sc((u - 1) // 2)

        # ---- B load: (c,sigma) partitions x (m1, t); 16 chunked reads
        B = const.tile([128, 64, T], bf16)
        for blk in range(8):
            for c in range(2):
                nc.gpsimd.dma_start(
                    B[c * 64 : (c + 1) * 64, blk * 8 : (blk + 1) * 8, :],
                    adram[c, blk * 8 : (blk + 1) * 8].rearrange(
                        "m s t -> s m t"
                    ),
                )

        # ---- stage 2 + PSUM OLA: per m1, zero-sliver mm + 4 shifted mms
        for blk in range(8):
            xo = xpool.tile([16, 8, SPAD], bf16, tag="xout")
            for i in range(8):
                m1 = blk * 8 + i
                P = s2ps.tile([16, 1024], f32, tag="s2ps")
                nc.tensor.matmul(
                    P[:, T : T + 4],
                    wz[:, 0:16],
                    B[:, m1, 0:4],
                    start=True,
                    stop=False,
                    skip_group_check=True,
                )
                for r in range(4):
                    nc.tensor.matmul(
                        P[:, r : r + T],
                        w2_sb[:, m1, r, :],
                        B[:, m1, :],
                        start=(r == 0),
                        stop=(r == 3),
                        skip_group_check=True,
                    )
                if i % 2 == 0:
                    nc.scalar.copy(xo[:, i, 0:SB], P[:, 0:SB])
                else:
                    nc.vector.tensor_copy(xo[:, i, 0:SB], P[:, 0:SB])
            nc.sync.dma_start(outre_ap[blk], xo[:])


# ---------------------------------------------------------------- build + run
_CACHE = {}


def _build(T):
    import concourse.bacc as bacc
    import concourse.tile as tile
    import concourse.mybir as mybir

    dt = mybir.dt
    nc = bacc.Bacc("TRN2", target_bir_lowering=False, debug=False, num_devices=N_CORES)
    zin_t = nc.dram_tensor("zin", [NG, 128, T], dt.bfloat16, kind="ExternalInput")
    w1_t = nc.dram_tensor("w1", [128, 64, 128], dt.bfloat16, kind="ExternalInput")
    w2_t = nc.dram_tensor("w2", [128, 64, 4, 16], dt.bfloat16, kind="ExternalInput")
    wim_t = nc.dram_tensor("wim", [8, 1024], dt.bfloat16, kind="ExternalInput")
    outre_t = nc.dram_tensor(
        "outre", [8, 16, 8, SPAD], dt.bfloat16, kind="ExternalOutput"
    )
    outim_t = nc.dram_tensor(
        "outim", [SC, 128, 1024], dt.bfloat16, kind="ExternalOutput"
    )
    with tile.TileContext(nc) as tc:
        emit_kernel(
            tc, outre_t.ap(), outim_t.ap(), zin_t.ap(), w1_t.ap(), w2_t.ap(),
            wim_t.ap(), T,
        )
    nc.compile()
    return nc


def core_out_to_sig(outre, outim, T):
    """outre [8,16,8,SPAD] bf16 + outim [SC,128,1024] bf16 ->
    [2, (T+3)*1024] f32."""
    sb = T + 3
    a = outre[:, :, :, :sb].astype(np.float32)  # [blk, (par,ih), m1i, s]
    a = a.reshape(8, 2, 8, 8, sb)  # [blk, par, ih, m1i, s]
    re_pis = a.transpose(1, 0, 3, 2, 4).reshape(128, 8, sb)  # [p, ih, s]
    re = re_pis.transpose(2, 1, 0).reshape(-1, 1024)[:sb]  # [s, 128*ih+p]
    im = outim.astype(np.float32).reshape(-1, 1024)[:sb]
    return np.stack([re.reshape(-1), im.reshape(-1)])


def kernel(z, window):
    from concourse.bass_utils import run_bass_kernel_spmd

    z = np.asarray(z, dtype=np.float32)
    window = np.asarray(window, dtype=np.float32)
    assert z.shape == (2, FREQ, T_FRAMES)

    if "nc" not in _CACHE:
        _CACHE["nc"] = _build(T_CORE)
    nc = _CACHE["nc"]

    w1T, w2rT, wim = build_weights(window)
    in_maps = []
    for m in range(N_CORES):
        zc = z[:, :, m * T_CORE : (m + 1) * T_CORE]
        in_maps.append(
            {"zin": prep_z_core(zc), "w1": w1T, "w2": w2rT, "wim": wim}
        )
    res = run_bass_kernel_spmd(nc, in_maps, core_ids=list(range(N_CORES)))

    full = np.zeros((2, L_FULL), dtype=np.float32)
    span = (T_CORE + 3) * 1024
    for m in range(N_CORES):
        o = core_out_to_sig(res.results[m]["outre"], res.results[m]["outim"], T_CORE)
        full[:, m * T_CORE * HOP : m * T_CORE * HOP + span] += o
    out = full[:, N_FFT // 2 : L_FULL - N_FFT // 2]

    win = window.astype(np.float64)
    ws_start = win[0:1024] + win[1024:2048] + win[2048:3072]
    ws_end = win[1024:2048] + win[2048:3072] + win[3072:4096]
    out[:, :1024] *= ((3.0 / 4096.0) / ws_start).astype(np.float32)[None, :]
    out[:, -1024:] *= ((3.0 / 4096.0) / ws_end).astype(np.float32)[None, :]
    return out
